# revision 10
# baseline (speedup 1.0000x reference)
"""Trainium2 Bass kernel for nn_MultiHeadAttention_84645215469987.

Problem (B=4, S=2048, E=1024, H=16, D=64):
    q/k/v = proj(query/key/value); per-head attention WITHOUT max-subtraction
    (logits are small); scores = sum_h attn_h@v_h + (H-1)*sum_h mean_k(v_h);
    out = reshape(scores.T)[B,128,1024] @ Wo.T + bo.

Sharding: 8 cores = (batch b = core//2) x (head-half g = core%2, 8 heads each).
Each core computes its partial out-projection [128,1024]; host sums the two
half-head partials per batch and adds bo.

Per-core device program (bf16 matmuls, fp32 accumulation):
  - v-proj -> v tiles [sk,65] per (head, sk-block) with a ones column
    (col 64) so attn@v also accumulates the softmax denominator.
  - q/k proj computed transposed: qT/kT [d-major, s] so per-head logits
    matmul is lhsT=kT (K=d=64), rhs=qT -> logits^T [sk,sq] in PSUM.
    Even/odd heads sit on partitions 0:64 / 64:128 -> PE row-group packing.
  - exp on ACT straight from PSUM with scale=1/8, bf16 out.
  - attn@v: lhsT=v' [sk=128,65], rhs=exp [sk,sq=512], accumulated over 16
    sk-blocks in PSUM -> o'^T [65,512]; row 64 = denominator.
  - PE-transpose o'^T chunks -> [sq,65]; DVE reciprocal+scale -> scores[s,d].
  - out-proj: scores(bf16) rearranged as lhsT, rhs=Wo^T; uniform-softmax
    term added as a rank-1 fp32 matmul (u2 x rowsum(Wo)) into the same PSUM.
"""

import os
import time

import numpy as np
import ml_dtypes

import concourse.bass as bass
import concourse.bacc as bacc
import concourse.mybir as mybir
import concourse.tile as tile
from concourse.bass import ts
from concourse.masks import make_identity

BF16 = mybir.dt.bfloat16
F32 = mybir.dt.float32
AF = mybir.ActivationFunctionType
ALU = mybir.AluOpType

S = 2048          # sequence length
E = 1024          # embed dim
HE = 512          # per-core projection width (8 heads x 64)
D = 64            # head dim
NH = 8            # heads per core
NEB = 8           # e-blocks of 128
NSB = 16          # s-blocks of 128
SCALE = 0.125     # 1/sqrt(64)


def _build_nc(debug=False):
    nc = bacc.Bacc()
    xq = nc.dram_tensor("xq", [E, S], BF16, kind="ExternalInput")
    xk = nc.dram_tensor("xk", [E, S], BF16, kind="ExternalInput")
    xv = nc.dram_tensor("xv", [E, S], BF16, kind="ExternalInput")
    wq = nc.dram_tensor("wq", [E, HE], BF16, kind="ExternalInput")
    wk = nc.dram_tensor("wk", [E, HE], BF16, kind="ExternalInput")
    wv = nc.dram_tensor("wv", [E, HE], BF16, kind="ExternalInput")
    wo = nc.dram_tensor("wo", [E, E], BF16, kind="ExternalInput")
    wors = nc.dram_tensor("wors", [1, E], F32, kind="ExternalInput")
    ubias = nc.dram_tensor("ubias", [1, D], F32, kind="ExternalInput")
    bqh = nc.dram_tensor("bqh", [128, 4], F32, kind="ExternalInput")
    bkh = nc.dram_tensor("bkh", [128, 4], F32, kind="ExternalInput")
    out = nc.dram_tensor("out", [128, E], F32, kind="ExternalOutput")
    dbg = {}
    if debug:
        dbg["qT"] = nc.dram_tensor("dbg_qT", [128, 4, S], BF16, kind="ExternalOutput")
        dbg["kT"] = nc.dram_tensor("dbg_kT", [128, 4, S], BF16, kind="ExternalOutput")
        dbg["v"] = nc.dram_tensor("dbg_v", [128, NSB, NH, 65], BF16, kind="ExternalOutput")
        dbg["vsum"] = nc.dram_tensor("dbg_vsum", [1, HE], F32, kind="ExternalOutput")
        dbg["scores"] = nc.dram_tensor("dbg_scores", [128, 8, 128], F32, kind="ExternalOutput")
        dbg["o"] = nc.dram_tensor("dbg_o", [65, 512], F32, kind="ExternalOutput")
        dbg["ex"] = nc.dram_tensor("dbg_ex", [128, 512], BF16, kind="ExternalOutput")

    with tile.TileContext(nc) as tc:
        with (
            tc.tile_pool(name="big", bufs=1) as big,
            tc.tile_pool(name="consts", bufs=1) as consts,
            tc.tile_pool(name="wop", bufs=2) as wop,
            tc.tile_pool(name="expp", bufs=4) as expp,
            tc.tile_pool(name="ocpp", bufs=2) as ocpp,
            tc.tile_pool(name="smalls", bufs=3) as smalls,
            tc.tile_pool(name="ps_lg", bufs=3, space="PSUM") as ps_lg,
            tc.tile_pool(name="ps_o", bufs=2, space="PSUM") as ps_o,
            tc.tile_pool(name="ps_t", bufs=1, space="PSUM") as ps_t,
            tc.tile_pool(name="ps_vs", bufs=1, space="PSUM") as ps_vs,
        ):
            # ---- constants ----
            ident = consts.tile([128, 128], F32)
            make_identity(nc, ident[:])
            zeros_col = consts.tile([128, 1], F32)
            nc.vector.memset(zeros_col[:], 0.0)
            ones_col = consts.tile([128, 1], BF16)
            nc.vector.memset(ones_col[:], 1.0)
            zeros512 = consts.tile([128, 512], BF16)
            nc.vector.memset(zeros512[:], 0.0)
            scores = big.tile([128, 8, 128], F32)
            nc.gpsimd.memset(scores[:], 0.0)
            sc2 = scores[:].rearrange("p kb (d two) -> p kb d two", two=2)

            # ---- input DMAs (emission order ~ priority) ----
            def load_x(dram):
                t = big.tile([128, NEB, S], BF16, tag=dram.name + "_sb")
                nc.sync.dma_start(t[:], dram[:].rearrange("(eb p) s -> p eb s", p=128))
                return t

            def load_w(dram):
                t = big.tile([128, NEB, HE], BF16, tag=dram.name + "_sb")
                nc.sync.dma_start(t[:], dram[:].rearrange("(eb p) j -> p eb j", p=128))
                return t

            xv_sb = load_x(xv)
            wv_sb = load_w(wv)
            xq_sb = load_x(xq)
            wq_sb = load_w(wq)
            xk_sb = load_x(xk)
            wk_sb = load_w(wk)
            bq_sb = consts.tile([128, 4], F32)
            nc.sync.dma_start(bq_sb[:], bqh[:])
            bk_sb = consts.tile([128, 4], F32)
            nc.sync.dma_start(bk_sb[:], bkh[:])
            wors_sb = consts.tile([1, E], F32)
            nc.sync.dma_start(wors_sb[:], wors[:])
            ubias_sb = consts.tile([1, D], F32)
            nc.sync.dma_start(ubias_sb[:], ubias[:])

            # ---- phase 1: v projection (untransposed) + ones col + vsum ----
            v_sb = big.tile([128, NSB, NH, 65], BF16)
            nc.vector.memset(v_sb[:, :, :, 64:65], 1.0)
            vs_ps = ps_vs.tile([1, HE], F32)
            # claim + zero the whole vsum bank ONCE: start=True clears
            # has_written for the entire bank, so per-head subgroups must
            # not each use start=True (they'd wipe each other's columns).
            nc.tensor.matmul(vs_ps[:], ones_col[:], zeros512[:], start=True, stop=False)
            for t in range(NSB):
                pv = ps_lg.tile([128, HE], F32, tag="lg")
                for eb in range(NEB):
                    nc.tensor.matmul(
                        pv[:],
                        xv_sb[:, eb, ts(t, 128)],
                        wv_sb[:, eb, :],
                        start=(eb == 0),
                        stop=(eb == NEB - 1),
                    )
                for h in range(NH):
                    nc.vector.tensor_copy(v_sb[:, t, h, 0:64], pv[:, ts(h, 64)])
                # vsum[j] += sum_p v[p, j] (for the uniform-softmax term)
                for h in range(NH):
                    nc.tensor.matmul(
                        vs_ps[0:1, ts(h, 64)],
                        ones_col[:],
                        v_sb[:, t, h, 0:64],
                        start=False,
                        stop=(t == NSB - 1 and h == NH - 1),
                        skip_group_check=True,
                    )

            # ---- phases 2+3 per head pair hp ----
            qT_sb = big.tile([128, 4, S], BF16)
            kT_sb = big.tile([128, 4, S], BF16)
            for hp in range(4):
                # q/k projections for this head pair, transposed: [j, s]
                for x_sb, w_sb, b_sb, dst in (
                    (xq_sb, wq_sb, bq_sb, qT_sb),
                    (xk_sb, wk_sb, bk_sb, kT_sb),
                ):
                    for sb4 in range(4):
                        pq = ps_lg.tile([128, 512], F32, tag="lg")
                        for eb in range(NEB):
                            nc.tensor.matmul(
                                pq[:],
                                w_sb[:, eb, ts(hp, 128)],
                                x_sb[:, eb, ts(sb4, 512)],
                                start=(eb == 0),
                                stop=(eb == NEB - 1),
                            )
                        # cast to bf16 + add per-j bias
                        nc.vector.tensor_scalar(
                            dst[:, hp, ts(sb4, 512)],
                            pq[:],
                            b_sb[:, hp : hp + 1],
                            None,
                            ALU.add,
                        )

                # attention for heads (2hp, 2hp+1)
                for sqb in range(4):
                    o_ps = [
                        ps_o.tile([128, 512], F32, tag="o", name="o_e"),
                        ps_o.tile([128, 512], F32, tag="o", name="o_o"),
                    ]
                    for m in range(NSB):
                        for par in range(2):
                            pb = par * 64
                            h = hp * 2 + par
                            lg = ps_lg.tile([128, 512], F32, tag="lg")
                            nc.tensor.matmul(
                                lg[:],
                                kT_sb[pb : pb + 64, hp, ts(m, 128)],
                                qT_sb[pb : pb + 64, hp, ts(sqb, 512)],
                                start=True,
                                stop=True,
                            )
                            ex = expp.tile([128, 512], BF16, tag="ex")
                            nc.scalar.activation(
                                ex[:], lg[:], AF.Exp, bias=zeros_col[:], scale=SCALE
                            )
                            if debug and hp == 0 and sqb == 0 and par == 0 and m == 0:
                                nc.sync.dma_start(dbg["ex"][:], ex[:])
                            nc.tensor.matmul(
                                o_ps[par][0:65, :],
                                v_sb[:, m, h, :],
                                ex[:],
                                start=(m == 0),
                                stop=(m == NSB - 1),
                            )
                    # normalize + accumulate scores
                    for par in range(2):
                        ocp = ocpp.tile([65, 512], F32, tag="ocp")
                        nc.vector.tensor_copy(ocp[:], o_ps[par][0:65, :])
                        if debug and hp == 0 and sqb == 0 and par == 0:
                            nc.sync.dma_start(dbg["o"][:], ocp[:])
                        for c in range(4):
                            tp = ps_t.tile([128, 65], F32, tag="t")
                            nc.tensor.transpose(
                                tp[:], ocp[0:65, ts(c, 128)], ident[0:65, 0:65]
                            )
                            rc = smalls.tile([128, 1], F32, tag="rc")
                            nc.vector.reciprocal(rc[:], tp[:, 64:65])
                            tmp = smalls.tile([128, 64], F32, tag="tmp")
                            nc.vector.tensor_scalar(
                                tmp[:], tp[:, 0:64], rc[:], None, ALU.mult
                            )
                            tix = sqb * 4 + c
                            kb, sbh = tix % 8, tix // 8
                            dst_ap = sc2[:, kb, :, sbh : sbh + 1]
                            nc.vector.tensor_tensor(
                                out=dst_ap,
                                in0=dst_ap,
                                in1=tmp[:].rearrange("p (d one) -> p d one", one=1),
                                op=ALU.add,
                            )

            # ---- phase 4: output projection + rank-1 uniform term ----
            # u2row[i] = (15/2048)*sum_h vsum[h*64 + i//2] + ubias[i//2]
            vsum_sb = consts.tile([1, HE], F32)
            nc.vector.tensor_copy(vsum_sb[:], vs_ps[:])
            u64 = consts.tile([1, D], F32)
            nc.vector.tensor_reduce(
                u64[:],
                vsum_sb[:].rearrange("p (h d) -> p d h", h=NH),
                axis=mybir.AxisListType.X,
                op=ALU.add,
            )
            u64b = consts.tile([1, D], F32)
            nc.vector.tensor_scalar(u64b[:], u64[:], 15.0 / 2048.0, None, ALU.mult)
            u64c = consts.tile([1, D], F32)
            nc.vector.tensor_tensor(out=u64c[:], in0=u64b[:], in1=ubias_sb[:], op=ALU.add)
            u2row = consts.tile([1, 128], F32)
            u2v = u2row[:].rearrange("p (d two) -> p d two", two=2)
            u64c3 = u64c[:].rearrange("p (d one) -> p d one", one=1)
            nc.vector.tensor_copy(u2v[:, :, 0:1], u64c3[:])
            nc.vector.tensor_copy(u2v[:, :, 1:2], u64c3[:])

            scores_bf = big.tile([128, 8, 128], BF16)
            nc.vector.tensor_copy(scores_bf[:], scores[:])
            sc_ap = scores_bf

            opA = ps_o.tile([128, 512], F32, tag="o")
            opB = ps_o.tile([128, 512], F32, tag="o")
            for kb in range(8):
                wo_kb = wop.tile([128, E], BF16, tag="wo")
                nc.sync.dma_start(wo_kb[:], wo[ts(kb, 128), :])
                nc.tensor.matmul(
                    opA[:], sc_ap[:, kb, :], wo_kb[:, 0:512], start=(kb == 0), stop=False
                )
                nc.tensor.matmul(
                    opB[:], sc_ap[:, kb, :], wo_kb[:, 512:1024], start=(kb == 0), stop=False
                )
            nc.tensor.matmul(opA[:], u2row[:], wors_sb[:, 0:512], start=False, stop=True)
            nc.tensor.matmul(opB[:], u2row[:], wors_sb[:, 512:1024], start=False, stop=True)
            out_sb = big.tile([128, E], F32)
            nc.vector.tensor_copy(out_sb[:, 0:512], opA[:])
            nc.vector.tensor_copy(out_sb[:, 512:1024], opB[:])
            nc.sync.dma_start(out[:], out_sb[:])
            if debug:
                nc.sync.dma_start(dbg["qT"][:], qT_sb[:])
                nc.sync.dma_start(dbg["kT"][:], kT_sb[:])
                nc.sync.dma_start(dbg["v"][:], v_sb[:])
                nc.sync.dma_start(dbg["vsum"][:], vsum_sb[:])
                nc.sync.dma_start(dbg["scores"][:], scores[:])

    nc.compile()
    return nc


_RT = {}


def _get_runtime():
    """Build nc + a cached sharded jit callable (compiled once per process)."""
    if _RT:
        return _RT
    import jax
    from jax.sharding import Mesh, PartitionSpec
    from jax.experimental.shard_map import shard_map
    from concourse.bass2jax import (
        _bass_exec_p,
        install_neuronx_cc_hook,
        partition_id_tensor,
    )

    install_neuronx_cc_hook()
    nc = _build_nc(debug=bool(int(os.environ.get("KBG_DEBUG", "0"))))

    partition_name = nc.partition_id_tensor.name if nc.partition_id_tensor else None
    in_names = []
    out_names = []
    out_avals = []
    for alloc in nc.m.functions[0].allocations:
        if not isinstance(alloc, mybir.MemoryLocationSet):
            continue
        name = alloc.memorylocations[0].name
        if alloc.kind == "ExternalInput":
            if name != partition_name:
                in_names.append(name)
        elif alloc.kind == "ExternalOutput":
            out_names.append(name)
            out_avals.append(
                jax.core.ShapedArray(tuple(alloc.tensor_shape), mybir.dt.np(alloc.dtype))
            )
    all_names = in_names + out_names
    if partition_name is not None:
        all_names = all_names + [partition_name]

    def _body(*args):
        operands = list(args)
        if partition_name is not None:
            operands.append(partition_id_tensor())
        outs = _bass_exec_p.bind(
            *operands,
            out_avals=tuple(out_avals),
            in_names=tuple(all_names),
            out_names=tuple(out_names),
            lowering_input_output_aliases=(),
            sim_require_finite=True,
            sim_require_nnan=True,
            nc=nc,
        )
        return tuple(outs)

    n_cores = 8
    devices = jax.devices()[:n_cores]
    mesh = Mesh(np.asarray(devices), ("core",))
    n_in = len(in_names) + len(out_avals)
    sharded = jax.jit(
        shard_map(
            _body,
            mesh=mesh,
            in_specs=(PartitionSpec("core"),) * n_in,
            out_specs=(PartitionSpec("core"),) * len(out_names),
            check_rep=False,
        ),
        keep_unused=True,
    )
    _RT.update(
        nc=nc, in_names=in_names, out_names=out_names, out_avals=out_avals,
        sharded=sharded, n_cores=n_cores,
    )
    return _RT


def _prep_in_maps(query, key, value, Wq, bq, Wk, bk, Wv, bv, Wo, bo):
    """Host-side sharding: core c -> batch c//2, head-half c%2."""
    bf = ml_dtypes.bfloat16

    def tb(a):  # contiguous bf16
        return np.ascontiguousarray(a, dtype=np.float32).astype(bf)

    wo_t = tb(Wo.T)
    wors = np.ascontiguousarray(Wo, dtype=np.float32).sum(axis=1).reshape(1, E)
    xt = {}
    for b in range(4):
        xt[b] = (tb(query[b].T), tb(key[b].T), tb(value[b].T))
    halves = {}
    for g in range(2):
        sl = slice(g * HE, (g + 1) * HE)
        halves[g] = dict(
            wq=tb(Wq[sl, :].T),
            wk=tb(Wk[sl, :].T),
            wv=tb(Wv[sl, :].T),
            bqh=np.ascontiguousarray(
                np.asarray(bq[sl], dtype=np.float32).reshape(4, 128).T
            ),
            bkh=np.ascontiguousarray(
                np.asarray(bk[sl], dtype=np.float32).reshape(4, 128).T
            ),
            ubias=(
                16.0 * np.asarray(bv[sl], dtype=np.float32).reshape(NH, D).sum(axis=0)
            ).reshape(1, D),
        )
    in_maps = []
    for c in range(8):
        b, g = c // 2, c % 2
        m = dict(
            xq=xt[b][0], xk=xt[b][1], xv=xt[b][2],
            wo=wo_t, wors=wors,
            **halves[g],
        )
        in_maps.append(m)
    return in_maps


def _run(in_maps):
    rt = _get_runtime()
    per_core = [[np.asarray(m[nm]) for nm in rt["in_names"]] for m in in_maps]
    concat_in = [
        np.concatenate([per_core[c][i] for c in range(rt["n_cores"])], axis=0)
        for i in range(len(rt["in_names"]))
    ]
    concat_zeros = [
        np.zeros((rt["n_cores"] * a.shape[0], *a.shape[1:]), a.dtype)
        for a in rt["out_avals"]
    ]
    out_arrs = rt["sharded"](*concat_in, *concat_zeros)
    outs = {
        nm: np.asarray(out_arrs[i]).reshape(rt["n_cores"], *rt["out_avals"][i].shape)
        for i, nm in enumerate(rt["out_names"])
    }
    return outs


def kernel(query, key, value, Wq, bq, Wk, bk, Wv, bv, Wo, bo, num_heads):
    assert int(num_heads) == 16
    query = np.asarray(query, dtype=np.float32)
    key = np.asarray(key, dtype=np.float32)
    value = np.asarray(value, dtype=np.float32)
    in_maps = _prep_in_maps(query, key, value, Wq, bq, Wk, bk, Wv, bv, Wo, bo)
    res = _run(in_maps)["out"]
    bo = np.asarray(bo, dtype=np.float32)
    out = np.stack([res[2 * b] + res[2 * b + 1] + bo for b in range(4)])
    return out.astype(np.float32)


def run_timed(inputs, iters=5):
    """Repeat device execution with device-resident inputs; returns
    (output, per-iter wall seconds list)."""
    import jax

    rt = _get_runtime()
    in_maps = _prep_in_maps(
        np.asarray(inputs["query"], np.float32),
        np.asarray(inputs["key"], np.float32),
        np.asarray(inputs["value"], np.float32),
        inputs["Wq"], inputs["bq"], inputs["Wk"], inputs["bk"],
        inputs["Wv"], inputs["bv"], inputs["Wo"], inputs["bo"],
    )
    per_core = [[np.asarray(m[nm]) for nm in rt["in_names"]] for m in in_maps]
    concat_in = [
        np.concatenate([per_core[c][i] for c in range(rt["n_cores"])], axis=0)
        for i in range(len(rt["in_names"]))
    ]
    concat_zeros = [
        np.zeros((rt["n_cores"] * a.shape[0], *a.shape[1:]), a.dtype)
        for a in rt["out_avals"]
    ]
    from jax.sharding import Mesh, PartitionSpec, NamedSharding

    devices = jax.devices()[: rt["n_cores"]]
    mesh = Mesh(np.asarray(devices), ("core",))
    shd = NamedSharding(mesh, PartitionSpec("core"))
    args = [jax.device_put(a, shd) for a in concat_in] + [
        jax.device_put(a, shd) for a in concat_zeros
    ]
    # warmup
    out_arrs = rt["sharded"](*args)
    jax.block_until_ready(out_arrs)
    times = []
    for _ in range(iters):
        t0 = time.perf_counter()
        out_arrs = rt["sharded"](*args)
        jax.block_until_ready(out_arrs)
        times.append(time.perf_counter() - t0)
    res = np.asarray(out_arrs[0]).reshape(rt["n_cores"], 128, E)
    bo = np.asarray(inputs["bo"], np.float32)
    out = np.stack([res[2 * b] + res[2 * b + 1] + bo for b in range(4)]).astype(
        np.float32
    )
    return out, times


if __name__ == "__main__":
    _build_nc()
    print("build OK")


# revision 16
# speedup vs baseline: 166.2040x; 166.2040x over previous
"""Trainium2 Bass kernel for nn_MultiHeadAttention_84645215469987.

Problem (B=4, S=2048, E=1024, H=16, D=64):
    q/k/v = proj(query/key/value); per-head attention WITHOUT max-subtraction
    (logits are small); scores = sum_h attn_h@v_h + (H-1)*sum_h mean_k(v_h);
    out = reshape(scores.T)[B,128,1024] @ Wo.T + bo.

Sharding: 8 cores = (batch b = core//2) x (head-half g = core%2, 8 heads each).
Each core computes its partial out-projection [128,1024]; host sums the two
half-head partials per batch and adds bo.

Per-core device program (bf16 matmuls, fp32 accumulation):
  - v-proj -> v tiles [sk,65] per (head, sk-block) with a ones column
    (col 64) so attn@v also accumulates the softmax denominator.
  - q/k proj computed transposed: qT/kT [d-major, s] so per-head logits
    matmul is lhsT=kT (K=d=64), rhs=qT -> logits^T [sk,sq] in PSUM.
    Even/odd heads sit on partitions 0:64 / 64:128 -> PE row-group packing.
  - exp on ACT straight from PSUM with scale=1/8, bf16 out.
  - attn@v: lhsT=v' [sk=128,65], rhs=exp [sk,sq=512], accumulated over 16
    sk-blocks in PSUM -> o'^T [65,512]; row 64 = denominator.
  - PE-transpose o'^T chunks -> [sq,65]; DVE reciprocal+scale -> scores[s,d].
  - out-proj: scores(bf16) rearranged as lhsT, rhs=Wo^T; uniform-softmax
    term added as a rank-1 fp32 matmul (u2 x rowsum(Wo)) into the same PSUM.
"""

import os
import time

import numpy as np
import ml_dtypes

import concourse.bass as bass
import concourse.bacc as bacc
import concourse.mybir as mybir
import concourse.tile as tile
from concourse.bass import ts
from concourse.masks import make_identity

BF16 = mybir.dt.bfloat16
FP8 = mybir.dt.float8e4
F32 = mybir.dt.float32
USE_DR = bool(int(os.environ.get("KBG_DR", "1")))
AF = mybir.ActivationFunctionType
ALU = mybir.AluOpType

S = 2048          # sequence length
E = 1024          # embed dim
HE = 512          # per-core projection width (8 heads x 64)
D = 64            # head dim
NH = 8            # heads per core
NEB = 8           # e-blocks of 128
NSB = 16          # s-blocks of 128
SCALE = 0.125     # 1/sqrt(64)


def _build_nc(debug=False, loop_n=1):
    nc = bacc.Bacc()
    xq = nc.dram_tensor("xq", [E, S], BF16, kind="ExternalInput")
    xk = nc.dram_tensor("xk", [E, S], BF16, kind="ExternalInput")
    xv = nc.dram_tensor("xv", [E, S], BF16, kind="ExternalInput")
    wq = nc.dram_tensor("wq", [E, HE], BF16, kind="ExternalInput")
    wk = nc.dram_tensor("wk", [E, HE], BF16, kind="ExternalInput")
    wv = nc.dram_tensor("wv", [E, HE], BF16, kind="ExternalInput")
    wo = nc.dram_tensor("wo", [E, E], BF16, kind="ExternalInput")
    wors = nc.dram_tensor("wors", [1, E], F32, kind="ExternalInput")
    ubias = nc.dram_tensor("ubias", [1, D], F32, kind="ExternalInput")
    bqh = nc.dram_tensor("bqh", [128, 4], F32, kind="ExternalInput")
    bkh = nc.dram_tensor("bkh", [128, 4], F32, kind="ExternalInput")
    out = nc.dram_tensor("out", [128, E], F32, kind="ExternalOutput")
    dbg = {}
    if debug:
        dbg["qT"] = nc.dram_tensor("dbg_qT", [128, 4, S], BF16, kind="ExternalOutput")
        dbg["kT"] = nc.dram_tensor("dbg_kT", [128, 4, S], BF16, kind="ExternalOutput")
        dbg["v"] = nc.dram_tensor("dbg_v", [128, NSB // 2, NH, 2, 80], FP8, kind="ExternalOutput")
        dbg["vsum"] = nc.dram_tensor("dbg_vsum", [1, HE], F32, kind="ExternalOutput")
        dbg["scores"] = nc.dram_tensor("dbg_scores", [128, 8, 128], F32, kind="ExternalOutput")
        dbg["o"] = nc.dram_tensor("dbg_o", [65, 512], F32, kind="ExternalOutput")
        dbg["ex"] = nc.dram_tensor("dbg_ex", [128, 2, 512], FP8, kind="ExternalOutput")

    import contextlib

    with tile.TileContext(nc) as tc:
        loop_ctx = tc.For_i(0, loop_n, 1) if loop_n > 1 else contextlib.nullcontext()
        with (
            loop_ctx,
            tc.tile_pool(name="big", bufs=1) as big,
            tc.tile_pool(name="consts", bufs=1) as consts,
            tc.tile_pool(name="wop", bufs=2) as wop,
            tc.tile_pool(name="expp", bufs=4) as expp,
            tc.tile_pool(name="ocpp", bufs=2) as ocpp,
            tc.tile_pool(name="smalls", bufs=3) as smalls,
            tc.tile_pool(name="ps_lg", bufs=2, space="PSUM") as ps_lg,
            tc.tile_pool(name="ps_o", bufs=2, space="PSUM") as ps_o,
            tc.tile_pool(name="ps_t", bufs=1, space="PSUM") as ps_t,
            tc.tile_pool(name="ps_vs", bufs=1, space="PSUM") as ps_vs,
        ):
            # ---- constants ----
            ident = consts.tile([128, 128], F32)
            make_identity(nc, ident[:])
            zeros_col = consts.tile([128, 1], F32)
            nc.vector.memset(zeros_col[:], 0.0)
            ones_col = consts.tile([128, 1], FP8)
            nc.vector.memset(ones_col[:], 1.0)
            ones_bf = consts.tile([128, 1], BF16)
            nc.vector.memset(ones_bf[:], 1.0)
            zeros512 = consts.tile([128, 512], FP8)
            nc.vector.memset(zeros512[:], 0.0)
            warm = consts.tile([128, 1], F32)
            nc.scalar.activation(warm[:], zeros_col[:], AF.Exp, bias=zeros_col[:], scale=1.0)
            scores = big.tile([128, 8, 128], F32)
            nc.gpsimd.memset(scores[:], 0.0)
            sc2 = scores[:].rearrange("p kb (d two) -> p kb d two", two=2)

            # ---- input DMAs (emission order ~ priority) ----
            def load_x(dram):
                t = big.tile([128, NEB, S], BF16, tag=dram.name + "_sb")
                nc.sync.dma_start(t[:], dram[:].rearrange("(eb p) s -> p eb s", p=128))
                return t

            def load_w(dram):
                t = big.tile([128, NEB, HE], BF16, tag=dram.name + "_sb")
                nc.sync.dma_start(t[:], dram[:].rearrange("(eb p) j -> p eb j", p=128))
                return t

            bq_sb = consts.tile([128, 4], F32)
            nc.sync.dma_start(bq_sb[:], bqh[:])
            bk_sb = consts.tile([128, 4], F32)
            nc.sync.dma_start(bk_sb[:], bkh[:])
            wq_sb = load_w(wq)
            xq_sb = load_x(xq)
            wk_sb = load_w(wk)
            xk_sb = load_x(xk)
            wv_sb = load_w(wv)
            xv_sb = load_x(xv)
            wors_sb = consts.tile([1, E], F32)
            nc.sync.dma_start(wors_sb[:], wors[:])
            ubias_sb = consts.tile([1, D], F32)
            nc.sync.dma_start(ubias_sb[:], ubias[:])

            qT_sb = big.tile([128, 4, S], BF16)
            kT_sb = big.tile([128, 4, S], BF16)

            def qk_proj(hp):
                for x_sb, w_sb, b_sb, dst in (
                    (xq_sb, wq_sb, bq_sb, qT_sb),
                    (xk_sb, wk_sb, bk_sb, kT_sb),
                ):
                    for sb4 in range(4):
                        pq = ps_lg.tile([128, 512], F32, tag="lg", name="pq")
                        for eb in range(NEB):
                            nc.tensor.matmul(
                                pq[:],
                                w_sb[:, eb, ts(hp, 128)],
                                x_sb[:, eb, ts(sb4, 512)],
                                start=(eb == 0),
                                stop=(eb == NEB - 1),
                            )
                        nc.vector.tensor_scalar(
                            dst[:, hp, ts(sb4, 512)],
                            pq[:],
                            b_sb[:, hp : hp + 1],
                            None,
                            ALU.add,
                        )

            qk_proj(0)

            # ---- phase 1: v projection (untransposed) + ones col + vsum ----
            v_sb = big.tile([128, NSB // 2, NH, 2, 80], FP8)
            nc.vector.memset(v_sb[:, :, :, :, 64:65], 1.0)
            vs_ps = ps_vs.tile([1, HE], F32)
            # claim + zero the whole vsum bank ONCE: start=True clears
            # has_written for the entire bank, so per-head subgroups must
            # not each use start=True (they'd wipe each other's columns).
            zeros_bf = consts.tile([128, 512], BF16)
            nc.vector.memset(zeros_bf[:], 0.0)
            nc.tensor.matmul(vs_ps[:], ones_bf[:], zeros_bf[:], start=True, stop=False)
            for t in range(NSB):
                pv = ps_lg.tile([128, HE], F32, tag="lg")
                for eb in range(NEB):
                    nc.tensor.matmul(
                        pv[:],
                        xv_sb[:, eb, ts(t, 128)],
                        wv_sb[:, eb, :],
                        start=(eb == 0),
                        stop=(eb == NEB - 1),
                    )
                vstage = expp.tile([128, HE], BF16, tag="vstage", bufs=2)
                nc.vector.tensor_copy(vstage[:], pv[:])
                for h in range(NH):
                    nc.vector.tensor_copy(
                        v_sb[:, t // 2, h, t % 2, 0:64], pv[:, ts(h, 64)]
                    )
                # vsum[j] += sum_p v[p, j] (for the uniform-softmax term)
                for h in range(NH):
                    nc.tensor.matmul(
                        vs_ps[0:1, ts(h, 64)],
                        ones_bf[:],
                        vstage[:, ts(h, 64)],
                        start=False,
                        stop=(t == NSB - 1 and h == NH - 1),
                        skip_group_check=True,
                    )

            # ---- phases 2+3 per head pair hp ----
            for hp in range(4):
                if hp > 0:
                    qk_proj(hp)

                # attention for heads (2hp, 2hp+1)
                for sqb in range(4):
                    o_ps = [
                        ps_o.tile([128, 512], F32, tag="o", name="o_e"),
                        ps_o.tile([128, 512], F32, tag="o", name="o_o"),
                    ]

                    def emit_dr(exs, mp):
                        for par in range(2):
                            h = hp * 2 + par
                            if USE_DR:
                                nc.tensor.matmul(
                                    o_ps[par][0:65, :],
                                    v_sb[:, mp, h, :, 0:65],
                                    exs[par][:],
                                    start=(mp == 0),
                                    stop=(mp == NSB // 2 - 1),
                                    perf_mode=mybir.MatmulPerfMode.DoubleRow,
                                )
                            else:
                                for sub in range(2):
                                    nc.tensor.matmul(
                                        o_ps[par][0:65, :],
                                        v_sb[:, mp, h, sub, 0:65],
                                        exs[par][:, sub, :],
                                        start=(mp == 0 and sub == 0),
                                        stop=(mp == NSB // 2 - 1 and sub == 1),
                                    )

                    pend = None
                    for mp in range(NSB // 2):
                        lg_by_par = [
                            ps_lg.tile([128, 2, 512], F32, tag="lg", name=f"lg{par}")
                            for par in range(2)
                        ]
                        # adjacent same-sub pairs at row groups 0/64 pack on PE
                        for sub in range(2):
                            m = 2 * mp + sub
                            for par in range(2):
                                pb = par * 64
                                nc.tensor.matmul(
                                    lg_by_par[par][:, sub, :],
                                    kT_sb[pb : pb + 64, hp, ts(m, 128)],
                                    qT_sb[pb : pb + 64, hp, ts(sqb, 512)],
                                    start=True,
                                    stop=True,
                                )
                        # previous step's attn@v drains while this step's exps run
                        if pend is not None:
                            emit_dr(pend, mp - 1)
                        exs = []
                        for par in range(2):
                            ex = expp.tile([128, 2, 512], FP8, tag="ex", name=f"ex{par}")
                            nc.scalar.activation(
                                ex[:], lg_by_par[par][:], AF.Exp,
                                bias=zeros_col[:], scale=SCALE,
                            )
                            if debug and hp == 0 and sqb == 0 and par == 0 and mp == 0:
                                nc.sync.dma_start(dbg["ex"][:], ex[:])
                            exs.append(ex)
                        pend = exs
                    emit_dr(pend, NSB // 2 - 1)
                    # normalize + accumulate scores
                    for par in range(2):
                        ocp = ocpp.tile([65, 512], F32, tag="ocp")
                        nc.vector.tensor_copy(ocp[:], o_ps[par][0:65, :])
                        if debug and hp == 0 and sqb == 0 and par == 0:
                            nc.sync.dma_start(dbg["o"][:], ocp[:])
                        for c in range(4):
                            tp = ps_t.tile([128, 65], F32, tag="t")
                            nc.tensor.transpose(
                                tp[:], ocp[0:65, ts(c, 128)], ident[0:65, 0:65]
                            )
                            rc = smalls.tile([128, 1], F32, tag="rc")
                            nc.vector.reciprocal(rc[:], tp[:, 64:65])
                            tmp = smalls.tile([128, 64], F32, tag="tmp")
                            nc.vector.tensor_scalar(
                                tmp[:], tp[:, 0:64], rc[:], None, ALU.mult
                            )
                            tix = sqb * 4 + c
                            kb, sbh = tix % 8, tix // 8
                            dst_ap = sc2[:, kb, :, sbh : sbh + 1]
                            nc.vector.tensor_tensor(
                                out=dst_ap,
                                in0=dst_ap,
                                in1=tmp[:].rearrange("p (d one) -> p d one", one=1),
                                op=ALU.add,
                            )

            # ---- phase 4: output projection + rank-1 uniform term ----
            # u2row[i] = (15/2048)*sum_h vsum[h*64 + i//2] + ubias[i//2]
            vsum_sb = consts.tile([1, HE], F32)
            nc.vector.tensor_copy(vsum_sb[:], vs_ps[:])
            u64 = consts.tile([1, D], F32)
            nc.vector.tensor_reduce(
                u64[:],
                vsum_sb[:].rearrange("p (h d) -> p d h", h=NH),
                axis=mybir.AxisListType.X,
                op=ALU.add,
            )
            u64b = consts.tile([1, D], F32)
            nc.vector.tensor_scalar(u64b[:], u64[:], 15.0 / 2048.0, None, ALU.mult)
            u64c = consts.tile([1, D], F32)
            nc.vector.tensor_tensor(out=u64c[:], in0=u64b[:], in1=ubias_sb[:], op=ALU.add)
            u2row = consts.tile([1, 128], F32)
            u2v = u2row[:].rearrange("p (d two) -> p d two", two=2)
            u64c3 = u64c[:].rearrange("p (d one) -> p d one", one=1)
            nc.vector.tensor_copy(u2v[:, :, 0:1], u64c3[:])
            nc.vector.tensor_copy(u2v[:, :, 1:2], u64c3[:])

            scores_bf = big.tile([128, 8, 128], BF16)
            nc.vector.tensor_copy(scores_bf[:], scores[:])
            sc_ap = scores_bf

            opA = ps_o.tile([128, 512], F32, tag="o")
            opB = ps_o.tile([128, 512], F32, tag="o")
            for kb in range(8):
                wo_kb = wop.tile([128, E], BF16, tag="wo")
                nc.sync.dma_start(wo_kb[:], wo[ts(kb, 128), :])
                nc.tensor.matmul(
                    opA[:], sc_ap[:, kb, :], wo_kb[:, 0:512], start=(kb == 0), stop=False
                )
                nc.tensor.matmul(
                    opB[:], sc_ap[:, kb, :], wo_kb[:, 512:1024], start=(kb == 0), stop=False
                )
            nc.tensor.matmul(opA[:], u2row[:], wors_sb[:, 0:512], start=False, stop=True)
            nc.tensor.matmul(opB[:], u2row[:], wors_sb[:, 512:1024], start=False, stop=True)
            out_sb = big.tile([128, E], F32)
            nc.vector.tensor_copy(out_sb[:, 0:512], opA[:])
            nc.vector.tensor_copy(out_sb[:, 512:1024], opB[:])
            nc.sync.dma_start(out[:], out_sb[:])
            if debug:
                nc.sync.dma_start(dbg["qT"][:], qT_sb[:])
                nc.sync.dma_start(dbg["kT"][:], kT_sb[:])
                nc.sync.dma_start(dbg["v"][:], v_sb[:])
                nc.sync.dma_start(dbg["vsum"][:], vsum_sb[:])
                nc.sync.dma_start(dbg["scores"][:], scores[:])

    nc.compile()
    return nc


_RT = {}


def _get_runtime():
    """Build nc + a cached sharded jit callable (compiled once per process)."""
    if _RT:
        return _RT
    import jax
    from jax.sharding import Mesh, PartitionSpec
    from jax.experimental.shard_map import shard_map
    from concourse.bass2jax import (
        _bass_exec_p,
        install_neuronx_cc_hook,
        partition_id_tensor,
    )

    install_neuronx_cc_hook()
    nc = _build_nc(
        debug=bool(int(os.environ.get("KBG_DEBUG", "0"))),
        loop_n=int(os.environ.get("KBG_LOOP", "1")),
    )

    partition_name = nc.partition_id_tensor.name if nc.partition_id_tensor else None
    in_names = []
    out_names = []
    out_avals = []
    for alloc in nc.m.functions[0].allocations:
        if not isinstance(alloc, mybir.MemoryLocationSet):
            continue
        name = alloc.memorylocations[0].name
        if alloc.kind == "ExternalInput":
            if name != partition_name:
                in_names.append(name)
        elif alloc.kind == "ExternalOutput":
            out_names.append(name)
            out_avals.append(
                jax.core.ShapedArray(tuple(alloc.tensor_shape), mybir.dt.np(alloc.dtype))
            )
    all_names = in_names + out_names
    if partition_name is not None:
        all_names = all_names + [partition_name]

    def _body(*args):
        operands = list(args)
        if partition_name is not None:
            operands.append(partition_id_tensor())
        outs = _bass_exec_p.bind(
            *operands,
            out_avals=tuple(out_avals),
            in_names=tuple(all_names),
            out_names=tuple(out_names),
            lowering_input_output_aliases=(),
            sim_require_finite=True,
            sim_require_nnan=True,
            nc=nc,
        )
        return tuple(outs)

    n_cores = 8
    devices = jax.devices()[:n_cores]
    mesh = Mesh(np.asarray(devices), ("core",))
    n_in = len(in_names) + len(out_avals)
    sharded = jax.jit(
        shard_map(
            _body,
            mesh=mesh,
            in_specs=(PartitionSpec("core"),) * n_in,
            out_specs=(PartitionSpec("core"),) * len(out_names),
            check_rep=False,
        ),
        keep_unused=True,
    )
    _RT.update(
        nc=nc, in_names=in_names, out_names=out_names, out_avals=out_avals,
        sharded=sharded, n_cores=n_cores,
    )
    return _RT


def _prep_in_maps(query, key, value, Wq, bq, Wk, bk, Wv, bv, Wo, bo):
    """Host-side sharding: core c -> batch c//2, head-half c%2."""
    bf = ml_dtypes.bfloat16

    def tb(a):  # contiguous bf16
        return np.ascontiguousarray(a, dtype=np.float32).astype(bf)

    wo_t = tb(Wo.T)
    wors = np.ascontiguousarray(Wo, dtype=np.float32).sum(axis=1).reshape(1, E)
    xt = {}
    for b in range(4):
        xt[b] = (tb(query[b].T), tb(key[b].T), tb(value[b].T))
    halves = {}
    for g in range(2):
        sl = slice(g * HE, (g + 1) * HE)
        halves[g] = dict(
            wq=tb(Wq[sl, :].T),
            wk=tb(Wk[sl, :].T),
            wv=tb(Wv[sl, :].T),
            bqh=np.ascontiguousarray(
                np.asarray(bq[sl], dtype=np.float32).reshape(4, 128).T
            ),
            bkh=np.ascontiguousarray(
                np.asarray(bk[sl], dtype=np.float32).reshape(4, 128).T
            ),
            ubias=(
                16.0 * np.asarray(bv[sl], dtype=np.float32).reshape(NH, D).sum(axis=0)
            ).reshape(1, D),
        )
    in_maps = []
    for c in range(8):
        b, g = c // 2, c % 2
        m = dict(
            xq=xt[b][0], xk=xt[b][1], xv=xt[b][2],
            wo=wo_t, wors=wors,
            **halves[g],
        )
        in_maps.append(m)
    return in_maps


def _run(in_maps):
    rt = _get_runtime()
    per_core = [[np.asarray(m[nm]) for nm in rt["in_names"]] for m in in_maps]
    concat_in = [
        np.concatenate([per_core[c][i] for c in range(rt["n_cores"])], axis=0)
        for i in range(len(rt["in_names"]))
    ]
    concat_zeros = [
        np.zeros((rt["n_cores"] * a.shape[0], *a.shape[1:]), a.dtype)
        for a in rt["out_avals"]
    ]
    out_arrs = rt["sharded"](*concat_in, *concat_zeros)
    outs = {
        nm: np.asarray(out_arrs[i]).reshape(rt["n_cores"], *rt["out_avals"][i].shape)
        for i, nm in enumerate(rt["out_names"])
    }
    return outs


def kernel(query, key, value, Wq, bq, Wk, bk, Wv, bv, Wo, bo, num_heads):
    assert int(num_heads) == 16
    query = np.asarray(query, dtype=np.float32)
    key = np.asarray(key, dtype=np.float32)
    value = np.asarray(value, dtype=np.float32)
    in_maps = _prep_in_maps(query, key, value, Wq, bq, Wk, bk, Wv, bv, Wo, bo)
    res = _run(in_maps)["out"]
    bo = np.asarray(bo, dtype=np.float32)
    out = np.stack([res[2 * b] + res[2 * b + 1] + bo for b in range(4)])
    return out.astype(np.float32)


def run_timed(inputs, iters=5):
    """Repeat device execution with device-resident inputs; returns
    (output, per-iter wall seconds list)."""
    import jax

    rt = _get_runtime()
    in_maps = _prep_in_maps(
        np.asarray(inputs["query"], np.float32),
        np.asarray(inputs["key"], np.float32),
        np.asarray(inputs["value"], np.float32),
        inputs["Wq"], inputs["bq"], inputs["Wk"], inputs["bk"],
        inputs["Wv"], inputs["bv"], inputs["Wo"], inputs["bo"],
    )
    per_core = [[np.asarray(m[nm]) for nm in rt["in_names"]] for m in in_maps]
    concat_in = [
        np.concatenate([per_core[c][i] for c in range(rt["n_cores"])], axis=0)
        for i in range(len(rt["in_names"]))
    ]
    concat_zeros = [
        np.zeros((rt["n_cores"] * a.shape[0], *a.shape[1:]), a.dtype)
        for a in rt["out_avals"]
    ]
    from jax.sharding import Mesh, PartitionSpec, NamedSharding

    devices = jax.devices()[: rt["n_cores"]]
    mesh = Mesh(np.asarray(devices), ("core",))
    shd = NamedSharding(mesh, PartitionSpec("core"))
    args = [jax.device_put(a, shd) for a in concat_in] + [
        jax.device_put(a, shd) for a in concat_zeros
    ]
    # warmup
    out_arrs = rt["sharded"](*args)
    jax.block_until_ready(out_arrs)
    times = []
    for _ in range(iters):
        t0 = time.perf_counter()
        out_arrs = rt["sharded"](*args)
        jax.block_until_ready(out_arrs)
        times.append(time.perf_counter() - t0)
    res = np.asarray(out_arrs[0]).reshape(rt["n_cores"], 128, E)
    bo = np.asarray(inputs["bo"], np.float32)
    out = np.stack([res[2 * b] + res[2 * b + 1] + bo for b in range(4)]).astype(
        np.float32
    )
    return out, times


if __name__ == "__main__":
    _build_nc()
    print("build OK")
